# revision 4
# baseline (speedup 1.0000x reference)
"""Liquid Neural Network Trainium2 kernel.

Reference computation (per batch element b, per step s):
    u      = x @ W_in.T + b_in                    # input layer
    ie     = u @ W_ih.T                           # input projection
    h_next = (1 - dt/tau) * h + (dt/tau) * (tanh(h) @ W_hh.T + ie_s + bias)
    out_s  = tanh(h_next) @ W_out.T + b_out

Strategy (8-way data parallel over batch):
  * Host folds the two input matmuls into one:  ie' = x @ W_comb.T + b_comb
    with W_comb = diag(dt/tau) @ W_ih @ W_in (tiny weight algebra on host).
  * Each core gets 32 batch rows; x shipped pre-transposed as [I, S*BS]
    (token = (s, b), b fastest) so the input projection needs no on-chip
    transpose.
  * On chip, phase A computes ie' for a chunk of steps (PE matmul + DVE
    bias-add), overlapped with the sequential scan.
  * The scan keeps tanh(h) as its state: for each step one PE matmul
    accumulates W_scan.T @ th into a PSUM bank pre-loaded with ie' (identity
    matmul injection, 16 steps per bank), then ScalarE computes
    th_next = tanh(psum) back into SBUF.  ie'-injection, output projection
    (W_out @ th, one matmul per 16 steps) and phase A all hide in the PE idle
    gaps of the latency-bound scan.
  * Outputs stream out per 16-step group: PSUM -> DVE -> SBUF -> DMA, in
    (s, b) order; host transposes back to [B, S, 1] and adds b_out.
"""

import numpy as np

B, I, H = 256, 32, 64
S = 4096
NCORES = 8
BS = B // NCORES                      # 32 batch rows per core
GROUP = 16                            # scan steps per PSUM bank
GCOLS = GROUP * BS                    # 512 = one full PSUM bank (fp32)
CHUNK_STEPS = 256                     # steps of ie' computed per phase-A chunk
TOK_PER_MM = 512                      # phase-A matmul moving-operand width

_nc_cache = {}


class _null:
    def __enter__(self):
        return None

    def __exit__(self, *a):
        return False


def _build(general: bool, n_steps: int):
    """Build (and compile) the per-core Bass program. Same NEFF on all cores."""
    import concourse.bacc as bacc
    import concourse.tile as tile
    from concourse import mybir

    ngroups = n_steps // GROUP
    chunk_steps = min(CHUNK_STEPS, n_steps)
    chunk_tok = chunk_steps * BS
    nchunks = n_steps // chunk_steps
    groups_per_chunk = chunk_steps // GROUP
    mms_per_chunk = chunk_tok // TOK_PER_MM
    ntok = n_steps * BS

    nc = bacc.Bacc(
        "TRN2",
        target_bir_lowering=False,
        debug=False,
        enable_asserts=False,
        num_devices=NCORES,
    )
    f32 = mybir.dt.float32
    Tanh = mybir.ActivationFunctionType.Tanh
    Copy = mybir.ActivationFunctionType.Copy

    x_d = nc.dram_tensor("x", [I, ntok], f32, kind="ExternalInput")
    scan_d = nc.dram_tensor("p_scan", [H, H], f32, kind="ExternalInput")
    comb_d = nc.dram_tensor("p_comb", [I, H], f32, kind="ExternalInput")
    wout_d = nc.dram_tensor("p_wout", [H, 1], f32, kind="ExternalInput")
    bcomb_d = nc.dram_tensor("p_bcomb", [H, 1], f32, kind="ExternalInput")
    if general:
        a_d = nc.dram_tensor("p_a", [H, H], f32, kind="ExternalInput")
    y_d = nc.dram_tensor("y", [ngroups, GCOLS], f32, kind="ExternalOutput")
    ident_d = nc.inline_tensor(np.eye(H, dtype=np.float32), name="ident")

    x_ap = x_d.ap()
    y_ap = y_d.ap()

    with tile.TileContext(nc) as tc:
        with (
            tc.tile_pool(name="consts", bufs=1) as consts,
            tc.tile_pool(name="xpool", bufs=2) as xpool,
            tc.tile_pool(name="iepool", bufs=2) as iepool,
            tc.tile_pool(name="thpool", bufs=3) as thpool,
            tc.tile_pool(name="opool", bufs=3) as opool,
            tc.tile_pool(name="psA", bufs=2, space="PSUM") as psApool,
            tc.tile_pool(name="psS", bufs=4, space="PSUM") as psSpool,
            tc.tile_pool(name="psO", bufs=2, space="PSUM") as psOpool,
            (tc.tile_pool(name="hpool", bufs=3) if general else _null()) as hpool,
        ):
            # --- constants into SBUF ---
            scan_sb = consts.tile([H, H], f32, name="scan_sb")
            nc.sync.dma_start(out=scan_sb, in_=scan_d.ap())
            comb_sb = consts.tile([I, H], f32, name="comb_sb")
            nc.sync.dma_start(out=comb_sb, in_=comb_d.ap())
            wout_sb = consts.tile([H, 1], f32, name="wout_sb")
            nc.sync.dma_start(out=wout_sb, in_=wout_d.ap())
            bcomb_sb = consts.tile([H, 1], f32, name="bcomb_sb")
            nc.sync.dma_start(out=bcomb_sb, in_=bcomb_d.ap())
            ident_sb = consts.tile([H, H], f32, name="ident_sb")
            nc.sync.dma_start(out=ident_sb, in_=ident_d.ap())
            if general:
                a_sb = consts.tile([H, H], f32, name="a_sb")
                nc.sync.dma_start(out=a_sb, in_=a_d.ap())
            th0 = consts.tile([H, BS], f32, name="th0")
            nc.vector.memset(th0, 0.0)
            if general:
                h0 = consts.tile([H, BS], f32, name="h0")
                nc.vector.memset(h0, 0.0)

            chunk_x = {}
            chunk_ie = {}
            th_tiles = {}
            h_tiles = {}
            proj_ps = {}

            def emit_chunk_dma(c):
                xt = xpool.tile([I, chunk_tok], f32, name=f"x_sb_{c}", tag="x")
                nc.sync.dma_start(
                    out=xt, in_=x_ap[:, c * chunk_tok : (c + 1) * chunk_tok]
                )
                chunk_x[c] = xt
                iet = iepool.tile([H, chunk_tok], f32, name=f"ie_sb_{c}", tag="ie")
                chunk_ie[c] = iet

            def emit_phase_a_mm(c, j):
                ps = psApool.tile([H, TOK_PER_MM], f32, name=f"psA_{c}_{j}", tag="psA")
                nc.tensor.matmul(
                    ps,
                    comb_sb,
                    chunk_x[c][:, j * TOK_PER_MM : (j + 1) * TOK_PER_MM],
                    start=True,
                    stop=True,
                )
                nc.vector.tensor_scalar_add(
                    out=chunk_ie[c][:, j * TOK_PER_MM : (j + 1) * TOK_PER_MM],
                    in0=ps,
                    scalar1=bcomb_sb,
                )

            def emit_proj(g):
                pso = psOpool.tile([1, GCOLS], f32, name=f"psO_{g}", tag="psO")
                nc.tensor.matmul(pso, wout_sb, th_tiles[g], start=True, stop=True)
                proj_ps[g] = pso

            def emit_out(g):
                osb = opool.tile([1, GCOLS], f32, name=f"osb_{g}", tag="o")
                nc.vector.tensor_copy(out=osb, in_=proj_ps[g])
                nc.sync.dma_start(out=y_ap[g : g + 1, :], in_=osb)
                del proj_ps[g]

            # --- phase A prologue: chunk 0 ---
            emit_chunk_dma(0)
            for j in range(mms_per_chunk):
                emit_phase_a_mm(0, j)

            # --- the scan ---
            for g in range(ngroups):
                c = g // groups_per_chunk
                gl = g % groups_per_chunk
                thb = thpool.tile([H, GCOLS], f32, name=f"th_{g}", tag="th")
                th_tiles[g] = thb
                if general:
                    hb = hpool.tile([H, GCOLS], f32, name=f"h_{g}", tag="h")
                    h_tiles[g] = hb
                ps = psSpool.tile([H, GCOLS], f32, name=f"psS_{g}", tag="psS")
                # pre-load this bank with ie' for all 16 steps of the group
                nc.tensor.matmul(
                    ps,
                    ident_sb,
                    chunk_ie[c][:, gl * GCOLS : (gl + 1) * GCOLS],
                    start=True,
                    stop=True,
                    skip_group_check=True,
                )
                for ds in range(GROUP):
                    s = g * GROUP + ds
                    # fillers, placed where PE sits idle waiting for tanh
                    if ds == 3 and g >= 1:
                        emit_proj(g - 1)
                    if ds == 5 and g >= 1:
                        emit_out(g - 1)
                    if ds == 8 and gl == 0 and c + 1 < nchunks:
                        emit_chunk_dma(c + 1)
                    if ds == 9 and c + 1 < nchunks and gl < mms_per_chunk:
                        emit_phase_a_mm(c + 1, gl)

                    if s == 0:
                        th_prev = th0
                    else:
                        pb, sl = (s - 1) // GROUP, (s - 1) % GROUP
                        th_prev = th_tiles[pb][:, sl * BS : (sl + 1) * BS]
                    last = ds == GROUP - 1
                    slot = ps[:, ds * BS : (ds + 1) * BS]
                    nc.tensor.matmul(
                        slot, scan_sb, th_prev, start=False,
                        stop=not general, skip_group_check=True,
                    )
                    if general:
                        if s == 0:
                            h_prev = h0
                        else:
                            pb, sl = (s - 1) // GROUP, (s - 1) % GROUP
                            h_prev = h_tiles[pb][:, sl * BS : (sl + 1) * BS]
                        nc.tensor.matmul(
                            slot, a_sb, h_prev, start=False, stop=True,
                            skip_group_check=True,
                        )
                    nc.scalar.activation(
                        out=thb[:, ds * BS : (ds + 1) * BS], in_=slot, func=Tanh
                    )
                    if general:
                        nc.scalar.activation(
                            out=hb[:, ds * BS : (ds + 1) * BS], in_=slot, func=Copy
                        )

            emit_proj(ngroups - 1)
            emit_out(ngroups - 1)

    nc.compile()
    return nc


def kernel(x, W_in, b_in, W_hh, W_ih, bias, tau, W_out, b_out):
    x = np.ascontiguousarray(np.asarray(x, dtype=np.float32))
    n_steps = x.shape[1]
    dt = 1.0
    tau64 = np.asarray(tau, np.float64)
    bscale = dt / tau64                               # dt/tau
    a = 1.0 - bscale
    general = bool(np.any(a != 0.0))

    W_in64 = np.asarray(W_in, np.float64)
    W_ih64 = np.asarray(W_ih, np.float64)
    W_hh64 = np.asarray(W_hh, np.float64)
    b_in64 = np.asarray(b_in, np.float64)
    bias64 = np.asarray(bias, np.float64)

    p_scan = np.ascontiguousarray(
        (bscale[:, None] * W_hh64).T.astype(np.float32)
    )                                                  # [H, H] lhsT for W_scan
    p_comb = np.ascontiguousarray(
        (bscale[:, None] * (W_ih64 @ W_in64)).T.astype(np.float32)
    )                                                  # [I, H] lhsT
    p_bcomb = (bscale * (W_ih64 @ b_in64 + bias64)).astype(np.float32).reshape(H, 1)
    p_wout = np.ascontiguousarray(np.asarray(W_out, np.float32).T)  # [H, 1]
    p_a = np.ascontiguousarray(np.diag(a).astype(np.float32))       # [H, H]

    key = (general, n_steps)
    if key not in _nc_cache:
        _nc_cache[key] = _build(general, n_steps)
    nc = _nc_cache[key]

    ntok = n_steps * BS
    in_maps = []
    for c in range(NCORES):
        xs = x[c * BS : (c + 1) * BS]                  # [BS, n_steps, I]
        xdev = np.ascontiguousarray(xs.transpose(2, 1, 0).reshape(I, ntok))
        m = {
            "x": xdev,
            "p_scan": p_scan,
            "p_comb": p_comb,
            "p_wout": p_wout,
            "p_bcomb": p_bcomb,
        }
        if general:
            m["p_a"] = p_a
        in_maps.append(m)

    from concourse.bass_utils import run_bass_kernel_spmd

    res = run_bass_kernel_spmd(nc, in_maps, core_ids=list(range(NCORES)))
    kernel.last_results = res

    nbatch = x.shape[0]
    y = np.empty((nbatch, n_steps, 1), np.float32)
    b_out_f = np.asarray(b_out, np.float32).reshape(-1)[0]
    for c in range(NCORES):
        yc = res.results[c]["y"].reshape(n_steps, BS)  # (s, b) order
        y[c * BS : (c + 1) * BS, :, 0] = yc.T
    y += b_out_f
    return y


kernel.last_results = None
